# revision 1
# baseline (speedup 1.0000x reference)
"""JANET 2-layer RNN kernel for 8 Trainium2 NeuronCores.

Strategy
--------
T=512, B=64, D_IN=512, H=1024.  The recurrent scan is inherently
sequential (1024 dependent steps) and is *weight-ingest bound* on the PE
array; per-step collectives have a ~5us floor which is worse than just
replicating the scan on every core.  So:

  phase P0: input projections for layer 0 (X @ ifW0.T etc.), sharded
            over T across the 8 cores, AllGather -> every core holds the
            full Pf0/Pg0 (biases folded in, bf16).
  phase S0: layer-0 scan, replicated on every core (bf16 weights so
            LDWEIGHTS uses fast-weight-load).  h kept in packed
            [128, 8*64] layout (H-chunk j on cols j*64..), fp32.
  phase P1: input projections for layer 1 from Y0, sharded over H_out
            (each core owns a 128-row slice via per-core weight inputs),
            AllGather.
  phase S1: layer-1 scan, replicated; h written to the fp32 output.

All per-core variation is pushed into the *input data* (per-core weight
slices / X slices) so the SPMD program is identical on all cores.
"""
import sys, os
sys.path.insert(0, '/opt/trn_rl_repo')
import numpy as np

from concourse import bass, bacc, tile
from concourse.bass_utils import run_bass_kernel_spmd

mybir = bass.mybir
dt = mybir.dt
AF = mybir.ActivationFunctionType
ALU = mybir.AluOpType

T, B, DIN, H = 512, 64, 512, 1024
BETA = 1.0
NCORE = 8
JC = H // 128          # 8 h-chunks
KIN = DIN // 128       # 4 k-tiles for layer-0 input proj
PACK = JC * B          # 512 packed cols for h


def build_program(T_steps=T, debug_taps=False):
    TBLK = T_steps // NCORE
    NTBB = TBLK * B            # per-core T-block cols
    NTB = T_steps * B
    nc = bacc.Bacc("TRN2", target_bir_lowering=False, debug=False,
                   num_devices=NCORE)

    bf16 = dt.bfloat16
    f32 = dt.float32

    # ---- inputs (per-core data) ----
    XT_c = nc.declare_dram_parameter("XT_c", [KIN, 128, NTBB], bf16, isOutput=False)
    W0T = nc.declare_dram_parameter("W0T", [2, KIN, 128, JC, 128], bf16, isOutput=False)
    H0T = nc.declare_dram_parameter("H0T", [2, JC, 128, JC, 128], bf16, isOutput=False)
    W1T_c = nc.declare_dram_parameter("W1T_c", [2, JC, 128, 128], bf16, isOutput=False)
    H1T = nc.declare_dram_parameter("H1T", [2, JC, 128, JC, 128], bf16, isOutput=False)
    B0 = nc.declare_dram_parameter("B0", [2, JC, 128, 1], f32, isOutput=False)
    B1_c = nc.declare_dram_parameter("B1_c", [2, 128, 1], f32, isOutput=False)
    Y1 = nc.declare_dram_parameter("Y1", [JC, 128, T_steps, B], f32, isOutput=True)

    # ---- internal DRAM ----
    PF0loc = nc.dram_tensor("PF0loc", [2, JC, 128, NTBB], bf16)
    PF0 = nc.dram_tensor("PF0", [NCORE, 2, JC, 128, NTBB], bf16, addr_space="Shared")
    Y0 = nc.dram_tensor("Y0", [JC, 128, T_steps, B], bf16)
    PF1loc = nc.dram_tensor("PF1loc", [2, 128, NTB], bf16)
    PF1 = nc.dram_tensor("PF1", [NCORE, 2, 128, NTB], bf16, addr_space="Shared")

    with tile.TileContext(nc) as tc:
        # ================= phase P0: layer-0 input projections ========
        with tc.tile_pool(name="p0_w", bufs=1) as wpool, \
             tc.tile_pool(name="p0_x", bufs=1) as xpool, \
             tc.tile_pool(name="p0_ps", bufs=4, space="PSUM") as pspool, \
             tc.tile_pool(name="p0_out", bufs=4) as opool, \
             tc.tile_pool(name="p0_b", bufs=1) as bpool:
            w_sb = wpool.tile([128, 2 * KIN * JC * 128], bf16)
            nc.sync.dma_start(w_sb[:], W0T.rearrange("g k p m q -> p g k m q"))
            x_sb = xpool.tile([128, KIN * NTBB], bf16)
            nc.sync.dma_start(x_sb[:], XT_c.rearrange("k p n -> p k n"))
            b_sb = bpool.tile([128, 2 * JC], f32)
            nc.sync.dma_start(b_sb[:], B0.rearrange("g m p o -> p g m o"))

            NCHUNK = min(512, NTBB)
            for g in range(2):
                for n in range(NTBB // NCHUNK):
                    for m in range(JC):
                        ps = pspool.tile([128, NCHUNK], f32, tag="ps")
                        for k in range(KIN):
                            nc.tensor.matmul(
                                ps[:],
                                w_sb[:, ((g * KIN + k) * JC + m) * 128:((g * KIN + k) * JC + m) * 128 + 128],
                                x_sb[:, k * NTBB + n * NCHUNK: k * NTBB + (n + 1) * NCHUNK],
                                start=(k == 0), stop=(k == KIN - 1))
                        ot = opool.tile([128, NCHUNK], bf16, tag="ot")
                        nc.scalar.activation(ot[:], ps[:], AF.Identity,
                                             bias=b_sb[:, g * JC + m: g * JC + m + 1])
                        nc.sync.dma_start(PF0loc.ap()[g, m, :, n * NCHUNK:(n + 1) * NCHUNK], ot[:])

        nc.gpsimd.collective_compute(
            "AllGather", ALU.bypass,
            ins=[PF0loc.ap().opt()], outs=[PF0.ap().opt()],
            replica_groups=[list(range(NCORE))])

        # ================= phase S0: layer-0 scan =====================
        scan_phase(nc, tc, T_steps, TBLK, H0T,
                   pf_view=PF0.rearrange("n g j p (t b) -> n g p j t b", b=B),
                   pf_has_blk=True, yout=Y0, ydt=bf16)

        # ================= phase P1: layer-1 input projections ========
        with tc.tile_pool(name="p1_w", bufs=1) as wpool, \
             tc.tile_pool(name="p1_x", bufs=6) as xpool, \
             tc.tile_pool(name="p1_ps", bufs=4, space="PSUM") as pspool, \
             tc.tile_pool(name="p1_out", bufs=4) as opool, \
             tc.tile_pool(name="p1_b", bufs=1) as bpool:
            w_sb = wpool.tile([128, 2 * JC * 128], bf16)
            nc.sync.dma_start(w_sb[:], W1T_c.rearrange("g k p q -> p g k q"))
            b_sb = bpool.tile([128, 2], f32)
            nc.sync.dma_start(b_sb[:], B1_c.rearrange("g p o -> p g o"))

            NCHUNK = 512
            TCH = NCHUNK // B  # 8 timesteps per chunk
            y0v = Y0.ap()
            for n in range(NTB // NCHUNK):
                rhs = xpool.tile([128, JC * NCHUNK], bf16, tag="rhs")
                for k in range(JC):
                    nc.sync.dma_start(rhs[:, k * NCHUNK:(k + 1) * NCHUNK],
                                      y0v[k, :, n * TCH:(n + 1) * TCH, :])
                for g in range(2):
                    ps = pspool.tile([128, NCHUNK], f32, tag="ps")
                    for k in range(JC):
                        nc.tensor.matmul(
                            ps[:],
                            w_sb[:, (g * JC + k) * 128:(g * JC + k) * 128 + 128],
                            rhs[:, k * NCHUNK:(k + 1) * NCHUNK],
                            start=(k == 0), stop=(k == JC - 1))
                    ot = opool.tile([128, NCHUNK], bf16, tag="ot")
                    nc.scalar.activation(ot[:], ps[:], AF.Identity,
                                         bias=b_sb[:, g:g + 1])
                    nc.sync.dma_start(PF1loc.ap()[g, :, n * NCHUNK:(n + 1) * NCHUNK], ot[:])

        nc.gpsimd.collective_compute(
            "AllGather", ALU.bypass,
            ins=[PF1loc.ap().opt()], outs=[PF1.ap().opt()],
            replica_groups=[list(range(NCORE))])

        # ================= phase S1: layer-1 scan =====================
        scan_phase(nc, tc, T_steps, TBLK, H1T,
                   pf_view=PF1.rearrange("n g p (t b) -> g p n t b", b=B),
                   pf_has_blk=False, yout=Y1, ydt=f32)

        if debug_taps:
            PF0dbg = nc.declare_dram_parameter(
                "PF0dbg", [NCORE, 2, JC, 128, NTBB], bf16, isOutput=True)
            Y0dbg = nc.declare_dram_parameter(
                "Y0dbg", [JC, 128, T_steps, B], bf16, isOutput=True)
            PF1dbg = nc.declare_dram_parameter(
                "PF1dbg", [NCORE, 2, 128, NTB], bf16, isOutput=True)
            PF0locdbg = nc.declare_dram_parameter(
                "PF0locdbg", [2, JC, 128, NTBB], bf16, isOutput=True)
            nc.sync.dma_start(PF0locdbg.ap()[:], PF0loc.ap()[:])
            nc.sync.dma_start(PF0dbg.ap()[:], PF0.ap()[:])
            nc.sync.dma_start(Y0dbg.ap()[:], Y0.ap()[:])
            nc.sync.dma_start(PF1dbg.ap()[:], PF1.ap()[:])

    nc.compile()
    return nc


def scan_phase(nc, tc, T_steps, TBLK, HT, pf_view, pf_has_blk, yout, ydt):
    """Replicated scan over T_steps.  h kept as two half tiles
    hA = chunks j=0..3 (cols 0..255), hB = chunks 4..7."""
    bf16 = dt.bfloat16
    f32 = dt.float32
    HALF = PACK // 2  # 256
    JH = JC // 2      # 4 chunks per half
    yv = yout.rearrange("j p t b -> p j t b")

    with tc.tile_pool(name="s_w", bufs=1) as wpool, \
         tc.tile_pool(name="s_pf", bufs=4) as pfpool, \
         tc.tile_pool(name="s_ps", bufs=2, space="PSUM") as pspool, \
         tc.tile_pool(name="s_h", bufs=3) as hpool, \
         tc.tile_pool(name="s_t", bufs=3) as tpool:
        w_sb = wpool.tile([128, 2 * JC * JC * 128], bf16)
        # layout: (g, k, m) -> col ((g*JC + k)*JC + m)*128
        nc.sync.dma_start(w_sb[:], HT.rearrange("g k p m q -> p g k m q"))

        hA = hpool.tile([128, HALF], f32, tag="hA")
        hB = hpool.tile([128, HALF], f32, tag="hB")
        hbA = hpool.tile([128, HALF], bf16, tag="hbA")
        hbB = hpool.tile([128, HALF], bf16, tag="hbB")
        nc.gpsimd.memset(hA[:], 0.0)
        nc.gpsimd.memset(hB[:], 0.0)
        nc.gpsimd.memset(hbA[:], 0.0)
        nc.gpsimd.memset(hbB[:], 0.0)

        def wslice(g, k, m):
            c = ((g * JC + k) * JC + m) * 128
            return w_sb[:, c:c + 128]

        for t in range(T_steps):
            blk, off = t // TBLK, t % TBLK
            pf = pfpool.tile([128, PACK], bf16, tag="pf")
            pg = pfpool.tile([128, PACK], bf16, tag="pg")
            if pf_has_blk:
                nc.sync.dma_start(pf[:], pf_view[blk, 0, :, :, off, :])
                nc.sync.dma_start(pg[:], pf_view[blk, 1, :, :, off, :])
            else:
                nc.sync.dma_start(pf[:], pf_view[0, :, :, t, :])
                nc.sync.dma_start(pg[:], pf_view[1, :, :, t, :])

            psFA = pspool.tile([128, HALF], f32, tag="psFA")
            psFB = pspool.tile([128, HALF], f32, tag="psFB")
            psGA = pspool.tile([128, HALF], f32, tag="psGA")
            psGB = pspool.tile([128, HALF], f32, tag="psGB")

            halves = ((psFA, psGA, hA, hbA, 0), (psFB, psGB, hB, hbB, JH))
            # all matmuls: F then G for half A, then half B
            for psF, psG, _, _, m0 in halves:
                for mi in range(JH):
                    m = m0 + mi
                    for k in range(JC):
                        nc.tensor.matmul(
                            psF[:, mi * B:(mi + 1) * B], wslice(0, k, m),
                            (hbA if k < JH else hbB)[:, (k % JH) * B:(k % JH + 1) * B],
                            start=(k == 0), stop=(k == JC - 1))
                for mi in range(JH):
                    m = m0 + mi
                    for k in range(JC):
                        nc.tensor.matmul(
                            psG[:, mi * B:(mi + 1) * B], wslice(1, k, m),
                            (hbA if k < JH else hbB)[:, (k % JH) * B:(k % JH + 1) * B],
                            start=(k == 0), stop=(k == JC - 1))

            newh = []
            for psF, psG, h, hb, m0 in halves:
                sl = slice(m0 * B, (m0 + JH) * B)
                fpre = tpool.tile([128, HALF], f32, tag="fpre")
                nc.vector.tensor_add(fpre[:], psF[:], pf[:, sl])
                F = tpool.tile([128, HALF], f32, tag="F")
                nc.scalar.activation(F[:], fpre[:], AF.Sigmoid)
                gpre = tpool.tile([128, HALF], f32, tag="gpre")
                nc.vector.tensor_add(gpre[:], psG[:], pg[:, sl])
                G = tpool.tile([128, HALF], f32, tag="G")
                nc.scalar.activation(G[:], gpre[:], AF.Tanh)
                d = tpool.tile([128, HALF], f32, tag="d")
                nc.vector.tensor_sub(d[:], h[:], G[:])
                xm = tpool.tile([128, HALF], f32, tag="xm")
                nc.vector.tensor_mul(xm[:], F[:], d[:])
                nh = hpool.tile([128, HALF], f32, tag="hA" if m0 == 0 else "hB")
                nc.vector.tensor_add(nh[:], G[:], xm[:])
                nhb = hpool.tile([128, HALF], bf16, tag="hbA" if m0 == 0 else "hbB")
                nc.scalar.activation(nhb[:], nh[:], AF.Identity)
                newh.append((nh, nhb, m0))

            for nh, nhb, m0 in newh:
                src = nh if ydt == f32 else nhb
                jstart = 0 if m0 == 0 else JH
                nc.sync.dma_start(yv[:, jstart:jstart + JH, t, :], src[:])

            hA, hB = newh[0][0], newh[1][0]
            hbA, hbB = newh[0][1], newh[1][1]


# ----------------------------------------------------------------------
# host-side wrapper
# ----------------------------------------------------------------------
_cached = {}


def _get_program(T_steps):
    if T_steps not in _cached:
        _cached[T_steps] = build_program(T_steps)
    return _cached[T_steps]


def _bf16(a):
    import ml_dtypes
    return np.asarray(a, np.float32).astype(ml_dtypes.bfloat16)


def make_in_maps(inputs, T_steps=T):
    TBLK = T_steps // NCORE
    X = np.asarray(inputs["X"], np.float32)[:T_steps]

    # XT_c per core: [KIN, 128, TBLK*B]  XT[d, t*B+b] = X[t, b, d]
    XT = np.ascontiguousarray(X.reshape(T_steps * B, DIN).T)  # [DIN, T*B]
    XT = XT.reshape(KIN, 128, T_steps, B)

    def wT(w):  # [out, in] -> [in, out] reshaped [k,128,m,128]
        wt = np.ascontiguousarray(np.asarray(w, np.float32).T)
        ki, ko = wt.shape
        return wt.reshape(ki // 128, 128, ko // 128, 128)

    W0T = _bf16(np.stack([wT(inputs["ifW0"]), wT(inputs["igW0"])]))
    H0T = _bf16(np.stack([wT(inputs["hfW0"]), wT(inputs["hgW0"])]))
    W1T = np.stack([wT(inputs["ifW1"]), wT(inputs["igW1"])])  # [2,8,128,8,128]
    H1T = _bf16(np.stack([wT(inputs["hfW1"]), wT(inputs["hgW1"])]))
    B0 = np.stack([
        (inputs["ifB0"] + inputs["hfB0"] - BETA).astype(np.float32),
        (inputs["igB0"] + inputs["hgB0"]).astype(np.float32),
    ]).reshape(2, JC, 128, 1)
    B1 = np.stack([
        (inputs["ifB1"] + inputs["hfB1"] - BETA).astype(np.float32),
        (inputs["igB1"] + inputs["hgB1"]).astype(np.float32),
    ]).reshape(2, JC, 128, 1)

    in_maps = []
    for c in range(NCORE):
        in_maps.append({
            "XT_c": _bf16(XT[:, :, c * TBLK:(c + 1) * TBLK, :].reshape(KIN, 128, TBLK * B)),
            "W0T": W0T,
            "H0T": H0T,
            "W1T_c": _bf16(W1T[:, :, :, c, :]),  # [2, 8, 128, 128]
            "H1T": H1T,
            "B0": B0,
            "B1_c": np.ascontiguousarray(B1[:, c]),
            "Y1": None,  # output
        })
        del in_maps[-1]["Y1"]
    return in_maps


def kernel(**inputs):
    T_steps = T
    nc = _get_program(T_steps)
    in_maps = make_in_maps(inputs, T_steps)
    res = run_bass_kernel_spmd(nc, in_maps, list(range(NCORE)))
    y = res.results[0]["Y1"]  # [JC, 128, T, B] fp32
    out = np.ascontiguousarray(y.transpose(2, 3, 0, 1).reshape(T_steps, B, H))
    return out



# revision 6
# speedup vs baseline: 1.0460x; 1.0460x over previous
"""JANET 2-layer RNN kernel for 8 Trainium2 NeuronCores.

Strategy
--------
T=512, B=64, D_IN=512, H=1024.  The recurrent scan is inherently
sequential (1024 dependent steps) and is *weight-ingest bound* on the PE
array; per-step collectives have a ~5us floor which is worse than just
replicating the scan on every core.  So:

  phase P0: input projections for layer 0 (X @ ifW0.T etc.), sharded
            over T across the 8 cores, AllGather -> every core holds the
            full Pf0/Pg0 (biases folded in, bf16).
  phase S0: layer-0 scan, replicated on every core (bf16 weights so
            LDWEIGHTS uses fast-weight-load).  h kept in packed
            [128, 8*64] layout (H-chunk j on cols j*64..), fp32.
  phase P1: input projections for layer 1 from Y0, sharded over H_out
            (each core owns a 128-row slice via per-core weight inputs),
            AllGather.
  phase S1: layer-1 scan, replicated; h written to the fp32 output.

All per-core variation is pushed into the *input data* (per-core weight
slices / X slices) so the SPMD program is identical on all cores.
"""
import sys, os
sys.path.insert(0, '/opt/trn_rl_repo')
import numpy as np

from concourse import bass, bacc, tile
from concourse.bass_utils import run_bass_kernel_spmd

mybir = bass.mybir
dt = mybir.dt
AF = mybir.ActivationFunctionType
ALU = mybir.AluOpType

T, B, DIN, H = 512, 64, 512, 1024
BETA = 1.0
NCORE = 8
JC = H // 128          # 8 h-chunks
KIN = DIN // 128       # 4 k-tiles for layer-0 input proj
PACK = JC * B          # 512 packed cols for h


def build_program(T_steps=T, debug_taps=False):
    TBLK = T_steps // NCORE
    NTBB = TBLK * B            # per-core T-block cols
    NTB = T_steps * B
    nc = bacc.Bacc("TRN2", target_bir_lowering=False, debug=False,
                   num_devices=NCORE)

    bf16 = dt.bfloat16
    f32 = dt.float32

    # ---- inputs (per-core data) ----
    EYE = nc.declare_dram_parameter("EYE", [128, 128], bf16, isOutput=False)
    XT_c = nc.declare_dram_parameter("XT_c", [KIN, 128, NTBB], bf16, isOutput=False)
    W0T = nc.declare_dram_parameter("W0T", [2, KIN, 128, JC, 128], bf16, isOutput=False)
    H0T = nc.declare_dram_parameter("H0T", [2, JC, 128, JC, 128], bf16, isOutput=False)
    W1T_c = nc.declare_dram_parameter("W1T_c", [2, JC, 128, 128], bf16, isOutput=False)
    H1T = nc.declare_dram_parameter("H1T", [2, JC, 128, JC, 128], bf16, isOutput=False)
    B0 = nc.declare_dram_parameter("B0", [2, JC, 128, 1], f32, isOutput=False)
    B1_c = nc.declare_dram_parameter("B1_c", [2, 128, 1], f32, isOutput=False)
    Y1 = nc.declare_dram_parameter("Y1", [JC, 128, T_steps, B], f32, isOutput=True)

    # ---- internal DRAM ----
    PF0loc = nc.dram_tensor("PF0loc", [2, JC, 128, NTBB], bf16)
    PF0 = nc.dram_tensor("PF0", [NCORE, 2, JC, 128, NTBB], bf16, addr_space="Shared")
    Y0 = nc.dram_tensor("Y0", [JC, 128, T_steps, B], bf16)
    PF1loc = nc.dram_tensor("PF1loc", [2, 128, NTB], bf16)
    PF1 = nc.dram_tensor("PF1", [NCORE, 2, 128, NTB], bf16, addr_space="Shared")

    with tile.TileContext(nc) as tc:
        # ================= phase P0: layer-0 input projections ========
        with tc.tile_pool(name="p0_w", bufs=1) as wpool, \
             tc.tile_pool(name="p0_x", bufs=1) as xpool, \
             tc.tile_pool(name="p0_ps", bufs=4, space="PSUM") as pspool, \
             tc.tile_pool(name="p0_out", bufs=4) as opool, \
             tc.tile_pool(name="p0_b", bufs=1) as bpool:
            w_sb = wpool.tile([128, 2 * KIN * JC * 128], bf16)
            nc.sync.dma_start(w_sb[:], W0T.rearrange("g k p m q -> p g k m q"))
            x_sb = xpool.tile([128, KIN * NTBB], bf16)
            nc.sync.dma_start(x_sb[:], XT_c.rearrange("k p n -> p k n"))
            b_sb = bpool.tile([128, 2 * JC], f32)
            nc.sync.dma_start(b_sb[:], B0.rearrange("g m p o -> p g m o"))

            NCHUNK = min(512, NTBB)
            for g in range(2):
                for n in range(NTBB // NCHUNK):
                    for m in range(JC):
                        ps = pspool.tile([128, NCHUNK], f32, tag="ps")
                        for k in range(KIN):
                            nc.tensor.matmul(
                                ps[:],
                                w_sb[:, ((g * KIN + k) * JC + m) * 128:((g * KIN + k) * JC + m) * 128 + 128],
                                x_sb[:, k * NTBB + n * NCHUNK: k * NTBB + (n + 1) * NCHUNK],
                                start=(k == 0), stop=(k == KIN - 1))
                        ot = opool.tile([128, NCHUNK], bf16, tag="ot")
                        nc.scalar.activation(ot[:], ps[:], AF.Identity,
                                             bias=b_sb[:, g * JC + m: g * JC + m + 1])
                        nc.sync.dma_start(PF0loc.ap()[g, m, :, n * NCHUNK:(n + 1) * NCHUNK], ot[:])

        nc.gpsimd.collective_compute(
            "AllGather", ALU.bypass,
            ins=[PF0loc.ap().opt()], outs=[PF0.ap().opt()],
            replica_groups=[list(range(NCORE))])

        # ================= phase S0: layer-0 scan =====================
        scan_phase(nc, tc, T_steps, TBLK, H0T, EYE,
                   pf_view=PF0.rearrange("n g j p (t b) -> n g p j t b", b=B),
                   pf_has_blk=True, yout=Y0, ydt=bf16)

        # ================= phase P1: layer-1 input projections ========
        with tc.tile_pool(name="p1_w", bufs=1) as wpool, \
             tc.tile_pool(name="p1_x", bufs=6) as xpool, \
             tc.tile_pool(name="p1_ps", bufs=4, space="PSUM") as pspool, \
             tc.tile_pool(name="p1_out", bufs=4) as opool, \
             tc.tile_pool(name="p1_b", bufs=1) as bpool:
            w_sb = wpool.tile([128, 2 * JC * 128], bf16)
            nc.sync.dma_start(w_sb[:], W1T_c.rearrange("g k p q -> p g k q"))
            b_sb = bpool.tile([128, 2], f32)
            nc.sync.dma_start(b_sb[:], B1_c.rearrange("g p o -> p g o"))

            NCHUNK = 512
            TCH = NCHUNK // B  # 8 timesteps per chunk
            y0v = Y0.ap()
            for n in range(NTB // NCHUNK):
                rhs = xpool.tile([128, JC * NCHUNK], bf16, tag="rhs")
                for k in range(JC):
                    nc.sync.dma_start(rhs[:, k * NCHUNK:(k + 1) * NCHUNK],
                                      y0v[k, :, n * TCH:(n + 1) * TCH, :])
                for g in range(2):
                    ps = pspool.tile([128, NCHUNK], f32, tag="ps")
                    for k in range(JC):
                        nc.tensor.matmul(
                            ps[:],
                            w_sb[:, (g * JC + k) * 128:(g * JC + k) * 128 + 128],
                            rhs[:, k * NCHUNK:(k + 1) * NCHUNK],
                            start=(k == 0), stop=(k == JC - 1))
                    ot = opool.tile([128, NCHUNK], bf16, tag="ot")
                    nc.scalar.activation(ot[:], ps[:], AF.Identity,
                                         bias=b_sb[:, g:g + 1])
                    nc.sync.dma_start(PF1loc.ap()[g, :, n * NCHUNK:(n + 1) * NCHUNK], ot[:])

        nc.gpsimd.collective_compute(
            "AllGather", ALU.bypass,
            ins=[PF1loc.ap().opt()], outs=[PF1.ap().opt()],
            replica_groups=[list(range(NCORE))])

        # ================= phase S1: layer-1 scan =====================
        scan_phase(nc, tc, T_steps, TBLK, H1T, EYE,
                   pf_view=PF1.rearrange("n g p (t b) -> g p n t b", b=B),
                   pf_has_blk=False, yout=Y1, ydt=f32)

        if debug_taps:
            PF0dbg = nc.declare_dram_parameter(
                "PF0dbg", [NCORE, 2, JC, 128, NTBB], bf16, isOutput=True)
            Y0dbg = nc.declare_dram_parameter(
                "Y0dbg", [JC, 128, T_steps, B], bf16, isOutput=True)
            PF1dbg = nc.declare_dram_parameter(
                "PF1dbg", [NCORE, 2, 128, NTB], bf16, isOutput=True)
            PF0locdbg = nc.declare_dram_parameter(
                "PF0locdbg", [2, JC, 128, NTBB], bf16, isOutput=True)
            nc.sync.dma_start(PF0locdbg.ap()[:], PF0loc.ap()[:])
            nc.sync.dma_start(PF0dbg.ap()[:], PF0.ap()[:])
            nc.sync.dma_start(Y0dbg.ap()[:], Y0.ap()[:])
            nc.sync.dma_start(PF1dbg.ap()[:], PF1.ap()[:])

    nc.compile()
    return nc


def scan_phase(nc, tc, T_steps, TBLK, HT, EYE, pf_view, pf_has_blk, yout, ydt):
    """Replicated scan over T_steps.

    Per-step structure (PE-stall-minimizing):
      - pf/pg are injected into PSUM via identity matmuls (no h dep, no
        DVE add on the critical path).
      - k-accumulation is split: all groups consume hbA (k=0..3) before
        any consume hbB (k=4..7), so the PE restarts the next step as
        soon as the first half of the new h exists.
      - PSUM readiness order is G-A, F-A, G-B, F-B so tanh starts early.
      - elementwise is spread: ACT does sigmoid/tanh, Pool does
        d=h-G and xm=F*d, DVE does nh(bf16) and nh(f32).
    """
    bf16 = dt.bfloat16
    f32 = dt.float32
    HALF = PACK // 2  # 256
    JH = JC // 2      # 4 chunks per half
    yv = yout.rearrange("j p t b -> p j t b")

    with tc.tile_pool(name="s_w", bufs=1) as wpool, \
         tc.tile_pool(name="s_pf", bufs=4) as pfpool, \
         tc.tile_pool(name="s_ps", bufs=2, space="PSUM") as pspool, \
         tc.tile_pool(name="s_h", bufs=3) as hpool, \
         tc.tile_pool(name="s_t", bufs=3) as tpool:
        w_sb = wpool.tile([128, 2 * JC * JC * 128], bf16)
        # layout: (g, k, m) -> col ((g*JC + k)*JC + m)*128
        nc.sync.dma_start(w_sb[:], HT.rearrange("g k p m q -> p g k m q"))
        eye_sb = wpool.tile([128, 128], bf16)
        nc.sync.dma_start(eye_sb[:], EYE.ap())

        hA = hpool.tile([128, HALF], f32, tag="hA")
        hB = hpool.tile([128, HALF], f32, tag="hB")
        hbA = hpool.tile([128, HALF], bf16, tag="hbA")
        hbB = hpool.tile([128, HALF], bf16, tag="hbB")
        nc.gpsimd.memset(hA[:], 0.0)
        nc.gpsimd.memset(hB[:], 0.0)
        nc.gpsimd.memset(hbA[:], 0.0)
        nc.gpsimd.memset(hbB[:], 0.0)

        def wslice(g, k, m):
            c = ((g * JC + k) * JC + m) * 128
            return w_sb[:, c:c + 128]

        for t in range(T_steps):
            blk, off = t // TBLK, t % TBLK
            pf = pfpool.tile([128, PACK], bf16, tag="pf")
            pg = pfpool.tile([128, PACK], bf16, tag="pg")
            if pf_has_blk:
                nc.sync.dma_start(pf[:], pf_view[blk, 0, :, :, off, :])
                nc.sync.dma_start(pg[:], pf_view[blk, 1, :, :, off, :])
            else:
                nc.sync.dma_start(pf[:], pf_view[0, :, :, t, :])
                nc.sync.dma_start(pg[:], pf_view[1, :, :, t, :])

            # one PSUM bank per half-gate group (padded to bank size)
            psFA = pspool.tile([128, HALF], f32, tag="psFA")
            psFB = pspool.tile([128, HALF], f32, tag="psFB")
            psGA = pspool.tile([128, HALF], f32, tag="psGA")
            psGB = pspool.tile([128, HALF], f32, tag="psGB")

            # -- injects: ps[m cols] = pf/pg (identity matmul, no h dep)
            for ps, src, m0 in ((psGA, pg, 0), (psGB, pg, JH),
                                (psFA, pf, 0), (psFB, pf, JH)):
                for mi in range(JH):
                    c = (m0 + mi) * B
                    nc.tensor.matmul(
                        ps[:, mi * B:(mi + 1) * B], eye_sb[:],
                        src[:, c:c + B], start=(mi == 0), stop=False)

            # -- k=0..3 (consume hbA) for every group
            for ps, g, m0 in ((psGA, 1, 0), (psGB, 1, JH),
                              (psFA, 0, 0), (psFB, 0, JH)):
                for mi in range(JH):
                    for k in range(JH):
                        nc.tensor.matmul(
                            ps[:, mi * B:(mi + 1) * B], wslice(g, k, m0 + mi),
                            hbA[:, k * B:(k + 1) * B], start=False, stop=False)

            # -- k=4..7 (consume hbB); order G-A, F-A, G-B, F-B
            for ps, g, m0 in ((psGA, 1, 0), (psFA, 0, 0),
                              (psGB, 1, JH), (psFB, 0, JH)):
                for mi in range(JH):
                    for k in range(JH):
                        nc.tensor.matmul(
                            ps[:, mi * B:(mi + 1) * B], wslice(g, JH + k, m0 + mi),
                            hbB[:, k * B:(k + 1) * B], start=False,
                            stop=(mi == JH - 1 and k == JH - 1))

            # -- activations (ACT), in PSUM readiness order
            GA = tpool.tile([128, HALF], f32, tag="GA")
            nc.scalar.activation(GA[:], psGA[:], AF.Tanh)
            FA = tpool.tile([128, HALF], f32, tag="FA")
            nc.scalar.activation(FA[:], psFA[:], AF.Sigmoid)
            GB = tpool.tile([128, HALF], f32, tag="GB")
            nc.scalar.activation(GB[:], psGB[:], AF.Tanh)
            FB = tpool.tile([128, HALF], f32, tag="FB")
            nc.scalar.activation(FB[:], psFB[:], AF.Sigmoid)

            # -- Pool engine: d = h - G, xm = F * d
            dA = tpool.tile([128, HALF], f32, tag="dA")
            nc.gpsimd.tensor_sub(dA[:], hA[:], GA[:])
            xmA = tpool.tile([128, HALF], f32, tag="xmA")
            nc.gpsimd.tensor_mul(xmA[:], FA[:], dA[:])
            dB = tpool.tile([128, HALF], f32, tag="dB")
            nc.gpsimd.tensor_sub(dB[:], hB[:], GB[:])
            xmB = tpool.tile([128, HALF], f32, tag="xmB")
            nc.gpsimd.tensor_mul(xmB[:], FB[:], dB[:])

            # -- DVE: new h in bf16 (critical, feeds next step's matmuls)
            #         then in f32 (feeds next step's d, and Y for layer 1)
            nhbA = hpool.tile([128, HALF], bf16, tag="hbA")
            nc.vector.tensor_add(nhbA[:], GA[:], xmA[:])
            nhbB = hpool.tile([128, HALF], bf16, tag="hbB")
            nc.vector.tensor_add(nhbB[:], GB[:], xmB[:])
            nhA = hpool.tile([128, HALF], f32, tag="hA")
            nc.vector.tensor_add(nhA[:], GA[:], xmA[:])
            nhB = hpool.tile([128, HALF], f32, tag="hB")
            nc.vector.tensor_add(nhB[:], GB[:], xmB[:])

            if ydt == f32:
                nc.sync.dma_start(yv[:, 0:JH, t, :], nhA[:])
                nc.sync.dma_start(yv[:, JH:JC, t, :], nhB[:])
            else:
                nc.sync.dma_start(yv[:, 0:JH, t, :], nhbA[:])
                nc.sync.dma_start(yv[:, JH:JC, t, :], nhbB[:])

            hA, hB, hbA, hbB = nhA, nhB, nhbA, nhbB


# ----------------------------------------------------------------------
# host-side wrapper
# ----------------------------------------------------------------------
_cached = {}


def _get_program(T_steps):
    if T_steps not in _cached:
        _cached[T_steps] = build_program(T_steps)
    return _cached[T_steps]


def _bf16(a):
    import ml_dtypes
    return np.asarray(a, np.float32).astype(ml_dtypes.bfloat16)


def make_in_maps(inputs, T_steps=T):
    TBLK = T_steps // NCORE
    X = np.asarray(inputs["X"], np.float32)[:T_steps]

    # XT_c per core: [KIN, 128, TBLK*B]  XT[d, t*B+b] = X[t, b, d]
    XT = np.ascontiguousarray(X.reshape(T_steps * B, DIN).T)  # [DIN, T*B]
    XT = XT.reshape(KIN, 128, T_steps, B)

    def wT(w):  # [out, in] -> [in, out] reshaped [k,128,m,128]
        wt = np.ascontiguousarray(np.asarray(w, np.float32).T)
        ki, ko = wt.shape
        return wt.reshape(ki // 128, 128, ko // 128, 128)

    W0T = _bf16(np.stack([wT(inputs["ifW0"]), wT(inputs["igW0"])]))
    H0T = _bf16(np.stack([wT(inputs["hfW0"]), wT(inputs["hgW0"])]))
    W1T = np.stack([wT(inputs["ifW1"]), wT(inputs["igW1"])])  # [2,8,128,8,128]
    H1T = _bf16(np.stack([wT(inputs["hfW1"]), wT(inputs["hgW1"])]))
    B0 = np.stack([
        (inputs["ifB0"] + inputs["hfB0"] - BETA).astype(np.float32),
        (inputs["igB0"] + inputs["hgB0"]).astype(np.float32),
    ]).reshape(2, JC, 128, 1)
    B1 = np.stack([
        (inputs["ifB1"] + inputs["hfB1"] - BETA).astype(np.float32),
        (inputs["igB1"] + inputs["hgB1"]).astype(np.float32),
    ]).reshape(2, JC, 128, 1)

    eye = _bf16(np.eye(128, dtype=np.float32))
    in_maps = []
    for c in range(NCORE):
        in_maps.append({
            "EYE": eye,
            "XT_c": _bf16(XT[:, :, c * TBLK:(c + 1) * TBLK, :].reshape(KIN, 128, TBLK * B)),
            "W0T": W0T,
            "H0T": H0T,
            "W1T_c": _bf16(W1T[:, :, :, c, :]),  # [2, 8, 128, 128]
            "H1T": H1T,
            "B0": B0,
            "B1_c": np.ascontiguousarray(B1[:, c]),
            "Y1": None,  # output
        })
        del in_maps[-1]["Y1"]
    return in_maps


def kernel(**inputs):
    T_steps = T
    nc = _get_program(T_steps)
    in_maps = make_in_maps(inputs, T_steps)
    res = run_bass_kernel_spmd(nc, in_maps, list(range(NCORE)))
    y = res.results[0]["Y1"]  # [JC, 128, T, B] fp32
    out = np.ascontiguousarray(y.transpose(2, 3, 0, 1).reshape(T_steps, B, H))
    return out



# revision 7
# speedup vs baseline: 1.2593x; 1.2040x over previous
"""JANET 2-layer RNN kernel for 8 Trainium2 NeuronCores.

Strategy
--------
T=512, B=64, D_IN=512, H=1024.  The recurrent scan is inherently
sequential (1024 dependent steps) and is *weight-ingest bound* on the PE
array; per-step collectives have a ~5us floor which is worse than just
replicating the scan on every core.  So:

  phase P0: input projections for layer 0 (X @ ifW0.T etc.), sharded
            over T across the 8 cores, AllGather -> every core holds the
            full Pf0/Pg0 (biases folded in, bf16).
  phase S0: layer-0 scan, replicated on every core (bf16 weights so
            LDWEIGHTS uses fast-weight-load).  h kept in packed
            [128, 8*64] layout (H-chunk j on cols j*64..), fp32.
  phase P1: input projections for layer 1 from Y0, sharded over H_out
            (each core owns a 128-row slice via per-core weight inputs),
            AllGather.
  phase S1: layer-1 scan, replicated; h written to the fp32 output.

All per-core variation is pushed into the *input data* (per-core weight
slices / X slices) so the SPMD program is identical on all cores.
"""
import sys, os
sys.path.insert(0, '/opt/trn_rl_repo')
import numpy as np

from concourse import bass, bacc, tile
from concourse.bass_utils import run_bass_kernel_spmd

mybir = bass.mybir
dt = mybir.dt
AF = mybir.ActivationFunctionType
ALU = mybir.AluOpType

T, B, DIN, H = 512, 64, 512, 1024
BETA = 1.0
NCORE = 8
JC = H // 128          # 8 h-chunks
KIN = DIN // 128       # 4 k-tiles for layer-0 input proj
PACK = JC * B          # 512 packed cols for h


def build_program(T_steps=T, debug_taps=False):
    TBLK = T_steps // NCORE
    NTBB = TBLK * B            # per-core T-block cols
    NTB = T_steps * B
    nc = bacc.Bacc("TRN2", target_bir_lowering=False, debug=False,
                   num_devices=NCORE)

    bf16 = dt.bfloat16
    f32 = dt.float32

    # ---- inputs (per-core data) ----
    EYE = nc.declare_dram_parameter("EYE", [128, 128], bf16, isOutput=False)
    XT_c = nc.declare_dram_parameter("XT_c", [KIN, 128, NTBB], bf16, isOutput=False)
    W0T = nc.declare_dram_parameter("W0T", [2, KIN, 128, JC, 128], bf16, isOutput=False)
    H0T = nc.declare_dram_parameter("H0T", [2, JC, 128, JC, 128], bf16, isOutput=False)
    W1T_c = nc.declare_dram_parameter("W1T_c", [2, JC, 128, 128], bf16, isOutput=False)
    H1T = nc.declare_dram_parameter("H1T", [2, JC, 128, JC, 128], bf16, isOutput=False)
    B0 = nc.declare_dram_parameter("B0", [2, JC, 128, 1], f32, isOutput=False)
    B1_c = nc.declare_dram_parameter("B1_c", [2, 128, 1], f32, isOutput=False)
    Y1 = nc.declare_dram_parameter("Y1", [JC, 128, T_steps, B], f32, isOutput=True)

    # ---- internal DRAM ----
    PF0loc = nc.dram_tensor("PF0loc", [2, JC, 128, NTBB], bf16)
    PF0 = nc.dram_tensor("PF0", [NCORE, 2, JC, 128, NTBB], bf16, addr_space="Shared")
    Y0 = nc.dram_tensor("Y0", [JC, 128, T_steps, B], bf16)
    PF1loc = nc.dram_tensor("PF1loc", [2, 128, NTB], bf16)
    PF1 = nc.dram_tensor("PF1", [NCORE, 2, 128, NTB], bf16, addr_space="Shared")

    with tile.TileContext(nc) as tc:
        # ================= phase P0: layer-0 input projections ========
        with tc.tile_pool(name="p0_w", bufs=1) as wpool, \
             tc.tile_pool(name="p0_x", bufs=1) as xpool, \
             tc.tile_pool(name="p0_ps", bufs=4, space="PSUM") as pspool, \
             tc.tile_pool(name="p0_out", bufs=4) as opool, \
             tc.tile_pool(name="p0_b", bufs=1) as bpool:
            w_sb = wpool.tile([128, 2 * KIN * JC * 128], bf16)
            nc.sync.dma_start(w_sb[:], W0T.rearrange("g k p m q -> p g k m q"))
            x_sb = xpool.tile([128, KIN * NTBB], bf16)
            nc.sync.dma_start(x_sb[:], XT_c.rearrange("k p n -> p k n"))
            b_sb = bpool.tile([128, 2 * JC], f32)
            nc.sync.dma_start(b_sb[:], B0.rearrange("g m p o -> p g m o"))

            NCHUNK = min(512, NTBB)
            for g in range(2):
                for n in range(NTBB // NCHUNK):
                    for m in range(JC):
                        ps = pspool.tile([128, NCHUNK], f32, tag="ps")
                        for k in range(KIN):
                            nc.tensor.matmul(
                                ps[:],
                                w_sb[:, ((g * KIN + k) * JC + m) * 128:((g * KIN + k) * JC + m) * 128 + 128],
                                x_sb[:, k * NTBB + n * NCHUNK: k * NTBB + (n + 1) * NCHUNK],
                                start=(k == 0), stop=(k == KIN - 1))
                        ot = opool.tile([128, NCHUNK], bf16, tag="ot")
                        nc.scalar.activation(ot[:], ps[:], AF.Identity,
                                             bias=b_sb[:, g * JC + m: g * JC + m + 1])
                        nc.sync.dma_start(PF0loc.ap()[g, m, :, n * NCHUNK:(n + 1) * NCHUNK], ot[:])

        nc.gpsimd.collective_compute(
            "AllGather", ALU.bypass,
            ins=[PF0loc.ap().opt()], outs=[PF0.ap().opt()],
            replica_groups=[list(range(NCORE))])

        # ================= phase S0: layer-0 scan =====================
        scan_phase(nc, tc, T_steps, TBLK, H0T, EYE,
                   pf_view=PF0.rearrange("n g j p (t b) -> n g p j t b", b=B),
                   pf_has_blk=True, yout=Y0, ydt=bf16)

        # ================= phase P1: layer-1 input projections ========
        with tc.tile_pool(name="p1_w", bufs=1) as wpool, \
             tc.tile_pool(name="p1_x", bufs=6) as xpool, \
             tc.tile_pool(name="p1_ps", bufs=4, space="PSUM") as pspool, \
             tc.tile_pool(name="p1_out", bufs=4) as opool, \
             tc.tile_pool(name="p1_b", bufs=1) as bpool:
            w_sb = wpool.tile([128, 2 * JC * 128], bf16)
            nc.sync.dma_start(w_sb[:], W1T_c.rearrange("g k p q -> p g k q"))
            b_sb = bpool.tile([128, 2], f32)
            nc.sync.dma_start(b_sb[:], B1_c.rearrange("g p o -> p g o"))

            NCHUNK = 512
            TCH = NCHUNK // B  # 8 timesteps per chunk
            y0v = Y0.ap()
            for n in range(NTB // NCHUNK):
                rhs = xpool.tile([128, JC * NCHUNK], bf16, tag="rhs")
                for k in range(JC):
                    nc.sync.dma_start(rhs[:, k * NCHUNK:(k + 1) * NCHUNK],
                                      y0v[k, :, n * TCH:(n + 1) * TCH, :])
                for g in range(2):
                    ps = pspool.tile([128, NCHUNK], f32, tag="ps")
                    for k in range(JC):
                        nc.tensor.matmul(
                            ps[:],
                            w_sb[:, (g * JC + k) * 128:(g * JC + k) * 128 + 128],
                            rhs[:, k * NCHUNK:(k + 1) * NCHUNK],
                            start=(k == 0), stop=(k == JC - 1))
                    ot = opool.tile([128, NCHUNK], bf16, tag="ot")
                    nc.scalar.activation(ot[:], ps[:], AF.Identity,
                                         bias=b_sb[:, g:g + 1])
                    nc.sync.dma_start(PF1loc.ap()[g, :, n * NCHUNK:(n + 1) * NCHUNK], ot[:])

        nc.gpsimd.collective_compute(
            "AllGather", ALU.bypass,
            ins=[PF1loc.ap().opt()], outs=[PF1.ap().opt()],
            replica_groups=[list(range(NCORE))])

        # ================= phase S1: layer-1 scan =====================
        scan_phase(nc, tc, T_steps, TBLK, H1T, EYE,
                   pf_view=PF1.rearrange("n g p (t b) -> g p n t b", b=B),
                   pf_has_blk=False, yout=Y1, ydt=f32)

        if debug_taps:
            PF0dbg = nc.declare_dram_parameter(
                "PF0dbg", [NCORE, 2, JC, 128, NTBB], bf16, isOutput=True)
            Y0dbg = nc.declare_dram_parameter(
                "Y0dbg", [JC, 128, T_steps, B], bf16, isOutput=True)
            PF1dbg = nc.declare_dram_parameter(
                "PF1dbg", [NCORE, 2, 128, NTB], bf16, isOutput=True)
            PF0locdbg = nc.declare_dram_parameter(
                "PF0locdbg", [2, JC, 128, NTBB], bf16, isOutput=True)
            nc.sync.dma_start(PF0locdbg.ap()[:], PF0loc.ap()[:])
            nc.sync.dma_start(PF0dbg.ap()[:], PF0.ap()[:])
            nc.sync.dma_start(Y0dbg.ap()[:], Y0.ap()[:])
            nc.sync.dma_start(PF1dbg.ap()[:], PF1.ap()[:])

    nc.compile()
    return nc


def scan_phase(nc, tc, T_steps, TBLK, HT, EYE, pf_view, pf_has_blk, yout, ydt):
    """Replicated scan over T_steps.

    Per-step structure (PE-stall-minimizing):
      - pf/pg are injected into PSUM via identity matmuls (no h dep, no
        DVE add on the critical path).
      - k-accumulation is split: all groups consume hbA (k=0..3) before
        any consume hbB (k=4..7), so the PE restarts the next step as
        soon as the first half of the new h exists.
      - PSUM readiness order is G-A, F-A, G-B, F-B so tanh starts early.
      - elementwise is spread: ACT does sigmoid/tanh, Pool does
        d=h-G and xm=F*d, DVE does nh(bf16) and nh(f32).
    """
    bf16 = dt.bfloat16
    f32 = dt.float32
    HALF = PACK // 2  # 256
    JH = JC // 2      # 4 chunks per half
    yv = yout.rearrange("j p t b -> p j t b")

    with tc.tile_pool(name="s_w", bufs=1) as wpool, \
         tc.tile_pool(name="s_pf", bufs=4) as pfpool, \
         tc.tile_pool(name="s_ps", bufs=2, space="PSUM") as pspool, \
         tc.tile_pool(name="s_h", bufs=3) as hpool, \
         tc.tile_pool(name="s_t", bufs=3) as tpool:
        w_sb = wpool.tile([128, 2 * JC * JC * 128], bf16)
        # layout: (g, k, m) -> col ((g*JC + k)*JC + m)*128
        nc.sync.dma_start(w_sb[:], HT.rearrange("g k p m q -> p g k m q"))
        eye_sb = wpool.tile([128, 128], bf16)
        nc.sync.dma_start(eye_sb[:], EYE.ap())

        hA = hpool.tile([128, HALF], f32, tag="hA")
        hB = hpool.tile([128, HALF], f32, tag="hB")
        hbA = hpool.tile([128, HALF], bf16, tag="hbA")
        hbB = hpool.tile([128, HALF], bf16, tag="hbB")
        nc.gpsimd.memset(hA[:], 0.0)
        nc.gpsimd.memset(hB[:], 0.0)
        nc.gpsimd.memset(hbA[:], 0.0)
        nc.gpsimd.memset(hbB[:], 0.0)

        def wslice(g, k, m):
            c = ((g * JC + k) * JC + m) * 128
            return w_sb[:, c:c + 128]

        for t in range(T_steps):
            blk, off = t // TBLK, t % TBLK
            pf = pfpool.tile([128, PACK], bf16, tag="pf")
            pg = pfpool.tile([128, PACK], bf16, tag="pg")
            if pf_has_blk:
                nc.sync.dma_start(pf[:], pf_view[blk, 0, :, :, off, :])
                nc.sync.dma_start(pg[:], pf_view[blk, 1, :, :, off, :])
            else:
                nc.sync.dma_start(pf[:], pf_view[0, :, :, t, :])
                nc.sync.dma_start(pg[:], pf_view[1, :, :, t, :])

            # one PSUM bank per half-gate group (padded to bank size)
            psFA = pspool.tile([128, HALF], f32, tag="psFA")
            psFB = pspool.tile([128, HALF], f32, tag="psFB")
            psGA = pspool.tile([128, HALF], f32, tag="psGA")
            psGB = pspool.tile([128, HALF], f32, tag="psGB")

            # -- injects: ps = pf/pg (identity matmul over the whole half,
            #    one 256-col matmul per bank; no h dep)
            nc.tensor.matmul(psGA[:], eye_sb[:], pg[:, 0:HALF],
                             start=True, stop=False)
            nc.tensor.matmul(psGB[:], eye_sb[:], pg[:, HALF:PACK],
                             start=True, stop=False)
            nc.tensor.matmul(psFA[:], eye_sb[:], pf[:, 0:HALF],
                             start=True, stop=False)
            nc.tensor.matmul(psFB[:], eye_sb[:], pf[:, HALF:PACK],
                             start=True, stop=False)

            # -- k=0..3 (consume hbA) for every group
            for ps, g, m0 in ((psGA, 1, 0), (psGB, 1, JH),
                              (psFA, 0, 0), (psFB, 0, JH)):
                for mi in range(JH):
                    for k in range(JH):
                        nc.tensor.matmul(
                            ps[:, mi * B:(mi + 1) * B], wslice(g, k, m0 + mi),
                            hbA[:, k * B:(k + 1) * B], start=False, stop=False)

            # -- k=4..7 (consume hbB); stop order G-A, F-A, G-B, F-B
            for ps, g, m0 in ((psGA, 1, 0), (psFA, 0, 0),
                              (psGB, 1, JH), (psFB, 0, JH)):
                for mi in range(JH):
                    for k in range(JH):
                        nc.tensor.matmul(
                            ps[:, mi * B:(mi + 1) * B], wslice(g, JH + k, m0 + mi),
                            hbB[:, k * B:(k + 1) * B], start=False,
                            stop=(mi == JH - 1 and k == JH - 1))

            # -- activations (ACT), in PSUM readiness order
            GA = tpool.tile([128, HALF], f32, tag="GA")
            nc.scalar.activation(GA[:], psGA[:], AF.Tanh)
            FA = tpool.tile([128, HALF], f32, tag="FA")
            nc.scalar.activation(FA[:], psFA[:], AF.Sigmoid)
            GB = tpool.tile([128, HALF], f32, tag="GB")
            nc.scalar.activation(GB[:], psGB[:], AF.Tanh)
            FB = tpool.tile([128, HALF], f32, tag="FB")
            nc.scalar.activation(FB[:], psFB[:], AF.Sigmoid)

            # -- DVE (fast engine) on the critical path to the next step's
            #    bf16 h: d = h - G, xm = F * d, nhb = G + xm
            dA = tpool.tile([128, HALF], f32, tag="dA")
            nc.vector.tensor_sub(dA[:], hA[:], GA[:])
            xmA = tpool.tile([128, HALF], f32, tag="xmA")
            nc.vector.tensor_mul(xmA[:], FA[:], dA[:])
            nhbA = hpool.tile([128, HALF], bf16, tag="hbA")
            nc.vector.tensor_add(nhbA[:], GA[:], xmA[:])
            dB = tpool.tile([128, HALF], f32, tag="dB")
            nc.vector.tensor_sub(dB[:], hB[:], GB[:])
            xmB = tpool.tile([128, HALF], f32, tag="xmB")
            nc.vector.tensor_mul(xmB[:], FB[:], dB[:])
            nhbB = hpool.tile([128, HALF], bf16, tag="hbB")
            nc.vector.tensor_add(nhbB[:], GB[:], xmB[:])

            # -- Pool engine (off critical path): f32 h for next step's d
            #    and for the layer-1 output
            nhA = hpool.tile([128, HALF], f32, tag="hA")
            nc.gpsimd.tensor_add(nhA[:], GA[:], xmA[:])
            nhB = hpool.tile([128, HALF], f32, tag="hB")
            nc.gpsimd.tensor_add(nhB[:], GB[:], xmB[:])

            if ydt == f32:
                nc.sync.dma_start(yv[:, 0:JH, t, :], nhA[:])
                nc.sync.dma_start(yv[:, JH:JC, t, :], nhB[:])
            else:
                nc.sync.dma_start(yv[:, 0:JH, t, :], nhbA[:])
                nc.sync.dma_start(yv[:, JH:JC, t, :], nhbB[:])

            hA, hB, hbA, hbB = nhA, nhB, nhbA, nhbB


# ----------------------------------------------------------------------
# host-side wrapper
# ----------------------------------------------------------------------
_cached = {}


def _get_program(T_steps):
    if T_steps not in _cached:
        _cached[T_steps] = build_program(T_steps)
    return _cached[T_steps]


def _bf16(a):
    import ml_dtypes
    return np.asarray(a, np.float32).astype(ml_dtypes.bfloat16)


def make_in_maps(inputs, T_steps=T):
    TBLK = T_steps // NCORE
    X = np.asarray(inputs["X"], np.float32)[:T_steps]

    # XT_c per core: [KIN, 128, TBLK*B]  XT[d, t*B+b] = X[t, b, d]
    XT = np.ascontiguousarray(X.reshape(T_steps * B, DIN).T)  # [DIN, T*B]
    XT = XT.reshape(KIN, 128, T_steps, B)

    def wT(w):  # [out, in] -> [in, out] reshaped [k,128,m,128]
        wt = np.ascontiguousarray(np.asarray(w, np.float32).T)
        ki, ko = wt.shape
        return wt.reshape(ki // 128, 128, ko // 128, 128)

    W0T = _bf16(np.stack([wT(inputs["ifW0"]), wT(inputs["igW0"])]))
    H0T = _bf16(np.stack([wT(inputs["hfW0"]), wT(inputs["hgW0"])]))
    W1T = np.stack([wT(inputs["ifW1"]), wT(inputs["igW1"])])  # [2,8,128,8,128]
    H1T = _bf16(np.stack([wT(inputs["hfW1"]), wT(inputs["hgW1"])]))
    B0 = np.stack([
        (inputs["ifB0"] + inputs["hfB0"] - BETA).astype(np.float32),
        (inputs["igB0"] + inputs["hgB0"]).astype(np.float32),
    ]).reshape(2, JC, 128, 1)
    B1 = np.stack([
        (inputs["ifB1"] + inputs["hfB1"] - BETA).astype(np.float32),
        (inputs["igB1"] + inputs["hgB1"]).astype(np.float32),
    ]).reshape(2, JC, 128, 1)

    eye = _bf16(np.eye(128, dtype=np.float32))
    in_maps = []
    for c in range(NCORE):
        in_maps.append({
            "EYE": eye,
            "XT_c": _bf16(XT[:, :, c * TBLK:(c + 1) * TBLK, :].reshape(KIN, 128, TBLK * B)),
            "W0T": W0T,
            "H0T": H0T,
            "W1T_c": _bf16(W1T[:, :, :, c, :]),  # [2, 8, 128, 128]
            "H1T": H1T,
            "B0": B0,
            "B1_c": np.ascontiguousarray(B1[:, c]),
            "Y1": None,  # output
        })
        del in_maps[-1]["Y1"]
    return in_maps


def kernel(**inputs):
    T_steps = T
    nc = _get_program(T_steps)
    in_maps = make_in_maps(inputs, T_steps)
    res = run_bass_kernel_spmd(nc, in_maps, list(range(NCORE)))
    y = res.results[0]["Y1"]  # [JC, 128, T, B] fp32
    out = np.ascontiguousarray(y.transpose(2, 3, 0, 1).reshape(T_steps, B, H))
    return out



# revision 9
# speedup vs baseline: 1.5135x; 1.2018x over previous
"""JANET 2-layer RNN kernel for 8 Trainium2 NeuronCores.

Strategy
--------
T=512, B=64, D_IN=512, H=1024.  The recurrent scan is inherently
sequential (1024 dependent steps); per-step collectives have a multi-us
floor which is worse than replicating the scan on every core.  So both
scans run replicated, and all projection work + data distribution is
pipelined around them:

  P0:  input projections for layer 0, sharded over H (each core computes
       its own 128-row slice for all T), emitted t-chunk-major with a
       chunked AllGather per 64-step chunk -> the scan starts after the
       first chunk lands (~0.1ms) instead of after a monolithic gather.
  S0:  layer-0 scan, replicated.  The layer-1 input projection for step
       t-1 rides inside step t's PE stream (16 extra 64-col matmuls
       consuming the same h tiles) -> no Y0 DRAM roundtrip at all; a
       chunked AllGather of the projections fires every 64 steps and
       completes while S0 is still running.
  S1:  layer-1 scan, replicated, reading the gathered chunks; writes the
       fp32 output.

Per scan step (PE-stall-minimizing):
  - pf/pg are injected into PSUM via one 256-col identity matmul per
    bank (start=True), so no DVE add sits on the critical path.
  - k-accumulation is split: all groups consume hbA (k=0..3) before any
    consume hbB, so the PE restarts the next step as soon as the first
    half of the new h exists.
  - PSUM stop order is G-A, F-A, G-B, F-B so tanh starts early.
  - DVE (fast) runs the critical chain d=h-G, xm=F*d, nhb=G+xm (bf16);
    ACT does sigmoid/tanh; the slow Pool engine only produces the
    off-critical-path f32 h.

All per-core variation is pushed into the *input data* (per-core weight
slices) so the SPMD program is identical on all cores.
"""
import sys, os
sys.path.insert(0, '/opt/trn_rl_repo')
import numpy as np

from concourse import bass, bacc, tile
from concourse.bass_utils import run_bass_kernel_spmd

mybir = bass.mybir
dt = mybir.dt
AF = mybir.ActivationFunctionType
ALU = mybir.AluOpType

T, B, DIN, H = 512, 64, 512, 1024
BETA = 1.0
NCORE = 8
JC = H // 128          # 8 h-chunks
KIN = DIN // 128       # 4 k-tiles for layer-0 input proj
PACK = JC * B          # 512 packed cols for h
NCH = 8                # t-chunks for pipelined AllGathers
HALF = PACK // 2       # 256
JH = JC // 2           # 4 chunks per half


def build_program(T_steps=T):
    CHS = T_steps // NCH       # steps per chunk
    CB = CHS * B               # cols per chunk
    NTB = T_steps * B
    nc = bacc.Bacc("TRN2", target_bir_lowering=False, debug=False,
                   num_devices=NCORE)

    bf16 = dt.bfloat16
    f32 = dt.float32

    # ---- inputs (per-core data) ----
    EYE = nc.declare_dram_parameter("EYE", [128, 128], bf16, isOutput=False)
    XT = nc.declare_dram_parameter("XT", [KIN, 128, NTB], bf16, isOutput=False)
    W0T_c = nc.declare_dram_parameter("W0T_c", [2, KIN, 128, 128], bf16, isOutput=False)
    B0_c = nc.declare_dram_parameter("B0_c", [2, 128, 1], f32, isOutput=False)
    H0T = nc.declare_dram_parameter("H0T", [2, JC, 128, JC, 128], bf16, isOutput=False)
    W1T_c = nc.declare_dram_parameter("W1T_c", [2, JC, 128, 128], bf16, isOutput=False)
    B1_c = nc.declare_dram_parameter("B1_c", [2, 128, 1], f32, isOutput=False)
    H1T = nc.declare_dram_parameter("H1T", [2, JC, 128, JC, 128], bf16, isOutput=False)
    Y1 = nc.declare_dram_parameter("Y1", [JC, 128, T_steps, B], f32, isOutput=True)

    # ---- internal DRAM (per-chunk, so AllGathers pipeline) ----
    PF0loc = [nc.dram_tensor(f"PF0loc{c}", [2, 128, CB], bf16) for c in range(NCH)]
    PF0g = [nc.dram_tensor(f"PF0g{c}", [NCORE, 2, 128, CB], bf16,
                           addr_space="Shared") for c in range(NCH)]
    PF1loc = [nc.dram_tensor(f"PF1loc{c}", [2, 128, CB], bf16) for c in range(NCH)]
    PF1g = [nc.dram_tensor(f"PF1g{c}", [NCORE, 2, 128, CB], bf16,
                           addr_space="Shared") for c in range(NCH)]

    RG = [list(range(NCORE))]

    with tile.TileContext(nc) as tc:
        # ============ phase P0: layer-0 input proj (H-sharded) ========
        with tc.tile_pool(name="p0_w", bufs=1) as wpool, \
             tc.tile_pool(name="p0_x", bufs=2) as xpool, \
             tc.tile_pool(name="p0_ps", bufs=4, space="PSUM") as pspool, \
             tc.tile_pool(name="p0_out", bufs=4) as opool, \
             tc.tile_pool(name="p0_b", bufs=1) as bpool:
            w_sb = wpool.tile([128, 2 * KIN * 128], bf16)
            nc.sync.dma_start(w_sb[:], W0T_c.rearrange("g k p q -> p g k q"))
            b_sb = bpool.tile([128, 2], f32)
            nc.sync.dma_start(b_sb[:], B0_c.rearrange("g p o -> p g o"))

            for ch in range(NCH):
                xc = xpool.tile([128, KIN * CB], bf16, tag="xc")
                for k in range(KIN):
                    nc.sync.dma_start(
                        xc[:, k * CB:(k + 1) * CB],
                        XT.ap()[k, :, ch * CB:(ch + 1) * CB])
                for n in range(CB // 512):
                    for g in range(2):
                        ps = pspool.tile([128, 512], f32, tag="ps")
                        for k in range(KIN):
                            nc.tensor.matmul(
                                ps[:],
                                w_sb[:, (g * KIN + k) * 128:(g * KIN + k) * 128 + 128],
                                xc[:, k * CB + n * 512:k * CB + (n + 1) * 512],
                                start=(k == 0), stop=(k == KIN - 1))
                        ot = opool.tile([128, 512], bf16, tag="ot")
                        nc.scalar.activation(ot[:], ps[:], AF.Identity,
                                             bias=b_sb[:, g:g + 1])
                        nc.sync.dma_start(
                            PF0loc[ch].ap()[g, :, n * 512:(n + 1) * 512], ot[:])
                nc.gpsimd.collective_compute(
                    "AllGather", ALU.bypass,
                    ins=[PF0loc[ch].ap().opt()], outs=[PF0g[ch].ap().opt()],
                    replica_groups=RG)

        # ============ phase S0: layer-0 scan + on-the-fly P1 ==========
        scan_phase(nc, tc, T_steps, H0T, EYE, pf_chunks=PF0g,
                   yout=None,
                   p1=dict(W1T_c=W1T_c, B1_c=B1_c, PF1loc=PF1loc,
                           PF1g=PF1g, RG=RG))

        # ============ phase S1: layer-1 scan ==========================
        scan_phase(nc, tc, T_steps, H1T, EYE, pf_chunks=PF1g,
                   yout=Y1, p1=None)

    nc.compile()
    return nc


def scan_phase(nc, tc, T_steps, HT, EYE, pf_chunks, yout, p1):
    bf16 = dt.bfloat16
    f32 = dt.float32
    CHS = T_steps // NCH
    yv = yout.rearrange("j p t b -> p j t b") if yout is not None else None
    pf_views = [pf.rearrange("n g p (t b) -> g p n t b", b=B)
                for pf in pf_chunks]

    with tc.tile_pool(name="s_w", bufs=1) as wpool, \
         tc.tile_pool(name="s_pf", bufs=4) as pfpool, \
         tc.tile_pool(name="s_ps", bufs=2, space="PSUM") as pspool, \
         tc.tile_pool(name="s_h", bufs=3) as hpool, \
         tc.tile_pool(name="s_t", bufs=3) as tpool:
        w_sb = wpool.tile([128, 2 * JC * JC * 128], bf16)
        # layout: (g, k, m) -> col ((g*JC + k)*JC + m)*128
        nc.sync.dma_start(w_sb[:], HT.rearrange("g k p m q -> p g k m q"))
        eye_sb = wpool.tile([128, 128], bf16)
        nc.sync.dma_start(eye_sb[:], EYE.ap())
        if p1 is not None:
            w1_sb = wpool.tile([128, 2 * JC * 128], bf16)
            nc.sync.dma_start(w1_sb[:], p1["W1T_c"].rearrange("g k p q -> p g k q"))
            b1_sb = wpool.tile([128, 2], f32)
            nc.sync.dma_start(b1_sb[:], p1["B1_c"].rearrange("g p o -> p g o"))

        hA = hpool.tile([128, HALF], f32, tag="hA")
        hB = hpool.tile([128, HALF], f32, tag="hB")
        hbA = hpool.tile([128, HALF], bf16, tag="hbA")
        hbB = hpool.tile([128, HALF], bf16, tag="hbB")
        nc.gpsimd.memset(hA[:], 0.0)
        nc.gpsimd.memset(hB[:], 0.0)
        nc.gpsimd.memset(hbA[:], 0.0)
        nc.gpsimd.memset(hbB[:], 0.0)

        def wslice(g, k, m):
            c = ((g * JC + k) * JC + m) * 128
            return w_sb[:, c:c + 128]

        def w1slice(g, k):
            c = (g * JC + k) * 128
            return w1_sb[:, c:c + 128]

        def p1_readout(t):
            """Bias + bf16 the p1 psum for step t, DMA to its chunk, and
            fire the chunk's AllGather when the chunk completes."""
            ch, toff = t // CHS, t % CHS
            p1o = tpool.tile([128, 2 * B], bf16, tag="p1o")
            for g in range(2):
                nc.scalar.activation(p1o[:, g * B:(g + 1) * B],
                                     p1ps[:, g * B:(g + 1) * B],
                                     AF.Identity, bias=b1_sb[:, g:g + 1])
                nc.sync.dma_start(
                    p1["PF1loc"][ch].ap()[g, :, toff * B:(toff + 1) * B],
                    p1o[:, g * B:(g + 1) * B])
            if toff == CHS - 1:
                nc.gpsimd.collective_compute(
                    "AllGather", ALU.bypass,
                    ins=[p1["PF1loc"][ch].ap().opt()],
                    outs=[p1["PF1g"][ch].ap().opt()],
                    replica_groups=p1["RG"])

        for t in range(T_steps):
            ch, toff = t // CHS, t % CHS
            pf = pfpool.tile([128, PACK], bf16, tag="pf")
            pg = pfpool.tile([128, PACK], bf16, tag="pg")
            nc.sync.dma_start(pf[:], pf_views[ch][0, :, :, toff, :])
            nc.sync.dma_start(pg[:], pf_views[ch][1, :, :, toff, :])

            # one PSUM bank per half-gate group (padded to bank size);
            # psGA's reader (tanh) finishes earliest -> bufs=1 is safe
            psFA = pspool.tile([128, HALF], f32, tag="psFA")
            psFB = pspool.tile([128, HALF], f32, tag="psFB")
            psGA = pspool.tile([128, HALF], f32, tag="psGA", bufs=1)
            psGB = pspool.tile([128, HALF], f32, tag="psGB")
            if p1 is not None and t > 0:
                p1ps = pspool.tile([128, 2 * B], f32, tag="p1ps", bufs=1)

            # -- injects: ps = pf/pg (identity matmul, no h dep)
            nc.tensor.matmul(psGA[:], eye_sb[:], pg[:, 0:HALF],
                             start=True, stop=False)
            nc.tensor.matmul(psGB[:], eye_sb[:], pg[:, HALF:PACK],
                             start=True, stop=False)
            nc.tensor.matmul(psFA[:], eye_sb[:], pf[:, 0:HALF],
                             start=True, stop=False)
            nc.tensor.matmul(psFB[:], eye_sb[:], pf[:, HALF:PACK],
                             start=True, stop=False)

            # -- k=0..3 (consume hbA) for every group
            for ps, g, m0 in ((psGA, 1, 0), (psGB, 1, JH),
                              (psFA, 0, 0), (psFB, 0, JH)):
                for mi in range(JH):
                    for k in range(JH):
                        nc.tensor.matmul(
                            ps[:, mi * B:(mi + 1) * B], wslice(g, k, m0 + mi),
                            hbA[:, k * B:(k + 1) * B], start=False, stop=False)
            # p1 for the *previous* h rides on the same hbA tiles
            if p1 is not None and t > 0:
                for g in range(2):
                    for k in range(JH):
                        nc.tensor.matmul(
                            p1ps[:, g * B:(g + 1) * B], w1slice(g, k),
                            hbA[:, k * B:(k + 1) * B],
                            start=(g == 0 and k == 0), stop=False)

            # -- k=4..7 (consume hbB); stop order G-A, F-A, G-B, F-B
            for ps, g, m0 in ((psGA, 1, 0), (psFA, 0, 0),
                              (psGB, 1, JH), (psFB, 0, JH)):
                for mi in range(JH):
                    for k in range(JH):
                        nc.tensor.matmul(
                            ps[:, mi * B:(mi + 1) * B], wslice(g, JH + k, m0 + mi),
                            hbB[:, k * B:(k + 1) * B], start=False,
                            stop=(mi == JH - 1 and k == JH - 1))
            if p1 is not None and t > 0:
                for g in range(2):
                    for k in range(JH):
                        nc.tensor.matmul(
                            p1ps[:, g * B:(g + 1) * B], w1slice(g, JH + k),
                            hbB[:, k * B:(k + 1) * B],
                            start=False, stop=(g == 1 and k == JH - 1))

            # -- activations (ACT), in PSUM readiness order
            GA = tpool.tile([128, HALF], f32, tag="GA")
            nc.scalar.activation(GA[:], psGA[:], AF.Tanh)
            FA = tpool.tile([128, HALF], f32, tag="FA")
            nc.scalar.activation(FA[:], psFA[:], AF.Sigmoid)
            GB = tpool.tile([128, HALF], f32, tag="GB")
            nc.scalar.activation(GB[:], psGB[:], AF.Tanh)
            FB = tpool.tile([128, HALF], f32, tag="FB")
            nc.scalar.activation(FB[:], psFB[:], AF.Sigmoid)

            # -- DVE on the critical path to the next step's bf16 h
            dA = tpool.tile([128, HALF], f32, tag="dA")
            nc.vector.tensor_sub(dA[:], hA[:], GA[:])
            xmA = tpool.tile([128, HALF], f32, tag="xmA")
            nc.vector.tensor_mul(xmA[:], FA[:], dA[:])
            nhbA = hpool.tile([128, HALF], bf16, tag="hbA")
            nc.vector.tensor_add(nhbA[:], GA[:], xmA[:])
            dB = tpool.tile([128, HALF], f32, tag="dB")
            nc.vector.tensor_sub(dB[:], hB[:], GB[:])
            xmB = tpool.tile([128, HALF], f32, tag="xmB")
            nc.vector.tensor_mul(xmB[:], FB[:], dB[:])
            nhbB = hpool.tile([128, HALF], bf16, tag="hbB")
            nc.vector.tensor_add(nhbB[:], GB[:], xmB[:])

            # -- Pool engine (off critical path): f32 h
            nhA = hpool.tile([128, HALF], f32, tag="hA")
            nc.gpsimd.tensor_add(nhA[:], GA[:], xmA[:])
            nhB = hpool.tile([128, HALF], f32, tag="hB")
            nc.gpsimd.tensor_add(nhB[:], GB[:], xmB[:])

            # -- p1 readout for step t-1 (psum accumulated above)
            if p1 is not None and t > 0:
                p1_readout(t - 1)

            if yout is not None:
                nc.sync.dma_start(yv[:, 0:JH, t, :], nhA[:])
                nc.sync.dma_start(yv[:, JH:JC, t, :], nhB[:])

            hA, hB, hbA, hbB = nhA, nhB, nhbA, nhbB

        # final p1: project the last h (consumes the final hb tiles)
        if p1 is not None:
            p1ps = pspool.tile([128, 2 * B], f32, tag="p1ps", bufs=1)
            for g in range(2):
                for k in range(JH):
                    nc.tensor.matmul(
                        p1ps[:, g * B:(g + 1) * B], w1slice(g, k),
                        hbA[:, k * B:(k + 1) * B],
                        start=(g == 0 and k == 0), stop=False)
            for g in range(2):
                for k in range(JH):
                    nc.tensor.matmul(
                        p1ps[:, g * B:(g + 1) * B], w1slice(g, JH + k),
                        hbB[:, k * B:(k + 1) * B],
                        start=False, stop=(g == 1 and k == JH - 1))
            p1_readout(T_steps - 1)


# ----------------------------------------------------------------------
# host-side wrapper
# ----------------------------------------------------------------------
_cached = {}


def _get_program(T_steps):
    if T_steps not in _cached:
        _cached[T_steps] = build_program(T_steps)
    return _cached[T_steps]


def _bf16(a):
    import ml_dtypes
    return np.asarray(a, np.float32).astype(ml_dtypes.bfloat16)


def make_in_maps(inputs, T_steps=T):
    X = np.asarray(inputs["X"], np.float32)[:T_steps]
    NTB = T_steps * B

    # XT: [KIN, 128, T*B]  XT[k, p, t*B+b] = X[t, b, k*128+p]
    XT = _bf16(np.ascontiguousarray(X.reshape(NTB, DIN).T).reshape(KIN, 128, NTB))

    def wT(w):  # [out, in] -> [in, out] reshaped [k,128,m,128]
        wt = np.ascontiguousarray(np.asarray(w, np.float32).T)
        ki, ko = wt.shape
        return wt.reshape(ki // 128, 128, ko // 128, 128)

    W0T = np.stack([wT(inputs["ifW0"]), wT(inputs["igW0"])])  # [2,KIN,128,8,128]
    H0T = _bf16(np.stack([wT(inputs["hfW0"]), wT(inputs["hgW0"])]))
    W1T = np.stack([wT(inputs["ifW1"]), wT(inputs["igW1"])])  # [2,8,128,8,128]
    H1T = _bf16(np.stack([wT(inputs["hfW1"]), wT(inputs["hgW1"])]))
    B0 = np.stack([
        (inputs["ifB0"] + inputs["hfB0"] - BETA).astype(np.float32),
        (inputs["igB0"] + inputs["hgB0"]).astype(np.float32),
    ]).reshape(2, JC, 128, 1)
    B1 = np.stack([
        (inputs["ifB1"] + inputs["hfB1"] - BETA).astype(np.float32),
        (inputs["igB1"] + inputs["hgB1"]).astype(np.float32),
    ]).reshape(2, JC, 128, 1)

    eye = _bf16(np.eye(128, dtype=np.float32))
    in_maps = []
    for c in range(NCORE):
        in_maps.append({
            "EYE": eye,
            "XT": XT,
            "W0T_c": _bf16(W0T[:, :, :, c, :]),   # [2, KIN, 128, 128]
            "B0_c": np.ascontiguousarray(B0[:, c]),
            "H0T": H0T,
            "W1T_c": _bf16(W1T[:, :, :, c, :]),   # [2, 8, 128, 128]
            "B1_c": np.ascontiguousarray(B1[:, c]),
            "H1T": H1T,
        })
    return in_maps


def kernel(**inputs):
    T_steps = T
    nc = _get_program(T_steps)
    in_maps = make_in_maps(inputs, T_steps)
    res = run_bass_kernel_spmd(nc, in_maps, list(range(NCORE)))
    y = res.results[0]["Y1"]  # [JC, 128, T, B] fp32
    out = np.ascontiguousarray(y.transpose(2, 3, 0, 1).reshape(T_steps, B, H))
    return out
